# revision 19
# baseline (speedup 1.0000x reference)
"""Trainium2 Bass kernel for GQA sliding-window attention.

Module: B=2, T=2048, D=2048, N=8 q-heads, K=4 kv-heads, H=256,
sliding window 1024, causal, RMSNorm on q/k, RoPE, softmax, out-proj.

Sharding (8 cores): batch (2) x kv-head (4).  Core c handles batch
c//4 and kv head c%4 (q heads 2*(c%4), 2*(c%4)+1).  Each core
produces a partial [T, D] output (its 2 heads' contribution); the
host sums the 4 partials per batch element.

All matmul operands are fp16 (PE runs 16-bit at full rate; fp32/fp32r
run at quarter rate on this hw).  Accumulation is always fp32 in PSUM,
softmax/rmsnorm statistics in fp32.  x is transposed on the host so
the contraction dim lands on partitions without on-device transposes.

Per-core device pipeline, per 128-row query tile i (16 tiles):
  A. DMA xT panel [128, ND, 128]; project q0|q1 and k|v (two N=512
     fp16 matmuls per d-chunk, accumulated over 16 d-chunks)
  B. RMSNorm q0/q1/k: ACT Square+accum_out (3x), one Sqrt, one DVE
     recip, fused scalar_tensor_tensor apply; v -> fp16 resident
  C. RoPE in fp16 (host cos/sin tables)
  D. PE-transpose q (4 tiles -> one PSUM bank -> 1 copy) and k
     (2 tiles -> 1 copy) -> qT (current), kT (resident [H, T])
  E. logits = qT.T @ kT over the sliding window (<=9 key blocks,
     fp16, <=512-col pieces), additive tri masks on edge blocks,
     ACT Exp(bias=-5) with accum_out -> probs(fp16) + row-sum l
  F. probs *= 1/l; PE-transpose probs blocks (batched 4/bank);
     PV: encT += v.T @ pT (fp16)
  G. encT -> SBUF (fp16)
  H. out[t-tile] = encT.T @ o_w (fp16, N=512), DMA PSUM -> DRAM.
"""

import numpy as np

B, T, D, N, K, H = 2, 2048, 2048, 8, 4, 256
P = 128
NT = T // P          # 16 query tiles
ND = D // P          # 16 contraction chunks
NB = 8               # max lookback blocks (window 1024 = 8 blocks)
WINDOW = 1024
SCALE = 0.0625
EPS = 1e-6
ROPE_BASE = 10000.0
NEG = -1e30
ESHIFT = 5.0         # exp(logit - 5): |logit|<=16 so max exp < 6e4 (fp16 safe)
NCORES = 8

_CACHE = {}


def _split_pieces(width):
    """Split width (multiple of 128) into <=512-col pieces."""
    n_p = -(-width // 512)
    pieces = []
    rem = width
    for _ in range(n_p - 1):
        w = max(128, (width // n_p // 128) * 128)
        pieces.append(w)
        rem -= w
    pieces.append(rem)
    return pieces


def _build_nc():
    import concourse.mybir as mybir
    import concourse.tile as tile
    from concourse import bacc
    from concourse.masks import make_causal_mask

    dt = mybir.dt
    f32, f16 = dt.float32, dt.float16
    MUL = mybir.AluOpType.mult
    ACTF = mybir.ActivationFunctionType

    nc = bacc.Bacc(
        "TRN2",
        target_bir_lowering=False,
        debug=False,
        enable_asserts=False,
        num_devices=NCORES,
    )

    xT_d = nc.dram_tensor("x", [D, T], f16, kind="ExternalInput").ap()
    wq_d = nc.dram_tensor("wq", [2, D, H], f16, kind="ExternalInput").ap()
    wk_d = nc.dram_tensor("wk", [D, H], f16, kind="ExternalInput").ap()
    wv_d = nc.dram_tensor("wv", [D, H], f16, kind="ExternalInput").ap()
    wo_d = nc.dram_tensor("wo", [2, H, D], f16, kind="ExternalInput").ap()
    idh_d = nc.dram_tensor("identh", [P, P], f16, kind="ExternalInput").ap()
    cos_d = nc.dram_tensor("cos", [T, H // 2], f16, kind="ExternalInput").ap()
    sin_d = nc.dram_tensor("sin", [T, H // 2], f16, kind="ExternalInput").ap()
    qs_d = nc.dram_tensor("qs", [H], f32, kind="ExternalInput").ap()
    ks_d = nc.dram_tensor("ks", [H], f32, kind="ExternalInput").ap()
    out_d = nc.dram_tensor("out", [T, D], f32, kind="ExternalOutput").ap()

    with tile.TileContext(nc) as tc:
        with (
            tc.tile_pool(name="consts", bufs=1) as consts,
            tc.tile_pool(name="ldx", bufs=3) as ldx,
            tc.tile_pool(name="work", bufs=3) as work,
            tc.tile_pool(name="ps_mix", bufs=3, space="PSUM") as ps_mix,
            tc.tile_pool(name="ps_proj", bufs=2, space="PSUM") as ps_proj,
            tc.tile_pool(name="ps_enc", bufs=1, space="PSUM") as ps_enc,
        ):
            # ---- constants / resident tensors ----
            ident_b = consts.tile([P, P], f16, name="ident_b")
            nc.scalar.dma_start(ident_b[:], idh_d[:])

            # mdiag: 0 where key<=query (lower tri incl diag), NEG above
            mdiag = consts.tile([P, P], f32, name="mdiag")
            make_causal_mask(nc, mdiag[:], mask_val=NEG)
            # mleft: NEG where key<=query, 0 above (strict upper allowed)
            mleft = consts.tile([P, P], f32, name="mleft")
            nc.gpsimd.memset(mleft[:], NEG)
            nc.gpsimd.affine_select(
                out=mleft[:],
                in_=mleft[:],
                compare_op=mybir.AluOpType.is_ge,
                fill=0.0,
                base=0,
                pattern=[[-1, P]],
                channel_multiplier=1,
            )

            eps_t = consts.tile([P, 1], f32, name="eps_t")
            nc.gpsimd.memset(eps_t[:], EPS)
            eshift_t = consts.tile([P, 1], f32, name="eshift_t")
            nc.gpsimd.memset(eshift_t[:], -ESHIFT)

            qs_t = consts.tile([P, H], f32, name="qs_t")
            nc.scalar.dma_start(qs_t[:], qs_d[None, :].broadcast_to([P, H]))
            ks_t = consts.tile([P, H], f32, name="ks_t")
            nc.scalar.dma_start(ks_t[:], ks_d[None, :].broadcast_to([P, H]))

            # weights: partition = d (within 128-chunk); per d-chunk the
            # two heads (or k|v) sit contiguously so one N=512 matmul
            # covers a full PSUM bank (one accumulation group per bank).
            wq_sb = consts.tile([P, ND, 2, H], f16, name="wq_sb")
            wkv_sb = consts.tile([P, ND, 2, H], f16, name="wkv_sb")
            wq_r = [wq_d[n].rearrange("(c p) h -> p c h", p=P) for n in range(2)]
            wk_r = wk_d.rearrange("(c p) h -> p c h", p=P)
            wv_r = wv_d.rearrange("(c p) h -> p c h", p=P)
            HD = ND // 2
            for dd in (0, HD):
                for n in range(2):
                    nc.scalar.dma_start(
                        wq_sb[:, dd:dd + HD, n, :], wq_r[n][:, dd:dd + HD, :]
                    )
                nc.scalar.dma_start(
                    wkv_sb[:, dd:dd + HD, 0, :], wk_r[:, dd:dd + HD, :]
                )
                nc.scalar.dma_start(
                    wkv_sb[:, dd:dd + HD, 1, :], wv_r[:, dd:dd + HD, :]
                )
            cos_sb = consts.tile([P, NT, H // 2], f16, name="cos_sb")
            nc.gpsimd.dma_start(cos_sb[:], cos_d.rearrange("(c p) f -> p c f", p=P))
            sin_sb = consts.tile([P, NT, H // 2], f16, name="sin_sb")
            nc.gpsimd.dma_start(sin_sb[:], sin_d.rearrange("(c p) f -> p c f", p=P))

            # wo: partition = h (within 128-chunk), [p, head, hc, D]
            wo_sb = consts.tile([P, 2, 2, D], f16, name="wo_sb")
            for n in range(2):
                nc.gpsimd.dma_start(
                    wo_sb[:, n, :, :],
                    wo_d[n].rearrange("(hc p) d -> p hc d", p=P),
                )

            # resident k/v for the whole sequence
            kT_sb = consts.tile([P, 2, T], f16, name="kT_sb")   # [h_in, hc, s]
            v_sb = consts.tile([P, NT, H], f16, name="v_sb")    # [s_in, sc, h]

            idh = ident_b[:]
            xT_r = xT_d.rearrange("(c p) t -> p c t", p=P)

            for i in range(NT):
                # ---- A: projections ----
                xp = ldx.tile([P, ND, P], f16, name="xp", tag="xp")
                nc.sync.dma_start(xp[:], xT_r[:, :, i * P:(i + 1) * P])
                proj = ps_proj.tile([P, 4 * H], f32, name="proj", tag="proj")
                for d in range(ND):
                    st, sp = (d == 0), (d == ND - 1)
                    nc.tensor.matmul(
                        proj[:, 0:2 * H], xp[:, d, :],
                        wq_sb[:, d, :, :], start=st, stop=sp,
                    )
                    nc.tensor.matmul(
                        proj[:, 2 * H:4 * H], xp[:, d, :],
                        wkv_sb[:, d, :, :], start=st, stop=sp,
                    )

                # ---- B: RMSNorm q0,q1,k ; v -> fp16 resident ----
                qkhat = work.tile([P, 3 * H], f16, name="qkhat", tag="qkhat")
                sqs = work.tile([P, H], f32, name="sqs", tag="sqs")
                ssq3 = work.tile([P, 3], f32, name="ssq3", tag="ssq3")
                for j in range(3):
                    nc.scalar.activation(
                        sqs[:], proj[:, j * H:(j + 1) * H], ACTF.Square,
                        accum_out=ssq3[:, j:j + 1],
                    )
                rms3 = work.tile([P, 3], f32, name="rms3", tag="rms3")
                nc.scalar.activation(
                    rms3[:], ssq3[:], ACTF.Sqrt, bias=eps_t[:], scale=1.0 / H
                )
                rinv3 = work.tile([P, 3], f32, name="rinv3", tag="rinv3")
                nc.vector.reciprocal(rinv3[:], rms3[:])
                for j, sc in ((0, qs_t), (1, qs_t), (2, ks_t)):
                    nc.vector.scalar_tensor_tensor(
                        qkhat[:, j * H:(j + 1) * H],
                        proj[:, j * H:(j + 1) * H],
                        rinv3[:, j:j + 1], sc[:],
                        op0=MUL, op1=MUL,
                    )
                nc.vector.tensor_copy(v_sb[:, i, :], proj[:, 3 * H:4 * H])

                # ---- C: RoPE (fp16, q0|q1|k together) ----
                qkrot = work.tile([P, 3 * H], f16, name="qkrot", tag="qkrot")
                F = H // 2
                cos_i = cos_sb[:, i, :]
                sin_i = sin_sb[:, i, :]
                cb = cos_i[:, None, :].broadcast_to([P, 3, F])
                sb = sin_i[:, None, :].broadcast_to([P, 3, F])
                q4 = qkhat[:].rearrange("p (n two f) -> p n two f", n=3, two=2)
                qr4 = qkrot[:].rearrange("p (n two f) -> p n two f", n=3, two=2)
                t1 = work.tile([P, 3, F], f16, name="t1", tag="t1")
                t2 = work.tile([P, 3, F], f16, name="t2", tag="t2")
                nc.vector.tensor_mul(t1[:], q4[:, :, 0, :], cb)
                nc.vector.tensor_mul(t2[:], q4[:, :, 1, :], sb)
                nc.vector.tensor_sub(qr4[:, :, 0, :], t1[:], t2[:])
                nc.vector.tensor_mul(t1[:], q4[:, :, 1, :], cb)
                nc.vector.tensor_mul(t2[:], q4[:, :, 0, :], sb)
                nc.vector.tensor_add(qr4[:, :, 1, :], t1[:], t2[:])

                # ---- D: transposes (q: 4 into one bank, k: 2) ----
                tq = ps_mix.tile([P, 512], f16, name="tq", tag="mix")
                for m in range(4):
                    nc.tensor.transpose(
                        tq[:, m * P:(m + 1) * P],
                        qkrot[:, m * P:(m + 1) * P], idh,
                    )
                qT = work.tile([P, 4, P], f16, name="qT", tag="qT")
                nc.vector.tensor_copy(qT[:], tq[:])
                tk = ps_mix.tile([P, 512], f16, name="tk", tag="mix")
                for hc in range(2):
                    nc.tensor.transpose(
                        tk[:, hc * P:(hc + 1) * P],
                        qkrot[:, 2 * H + hc * P:2 * H + (hc + 1) * P], idh,
                    )
                nc.vector.tensor_copy(
                    kT_sb[:, :, i * P:(i + 1) * P],
                    tk[:, 0:2 * P].rearrange("p (hc q) -> p hc q", hc=2),
                )

                # ---- E: logits/softmax per head ----
                j0 = max(0, i - NB)
                nblk = i - j0 + 1
                width = nblk * P
                s0 = j0 * P
                pieces = _split_pieces(width)
                encT = work.tile([P, 2 * H], f16, name="encT", tag="encT")
                probs2 = []
                for n in range(2):
                    probs = work.tile([P, 9 * P], f16, name="probs", tag="probs")
                    probs2.append(probs)
                    lacc = work.tile([P, 1], f32, name="lacc", tag="lacc")
                    off = 0
                    for pi, w in enumerate(pieces):
                        pl = ps_mix.tile([P, 512], f32, name="pl", tag="mix")
                        for hc in range(2):
                            nc.tensor.matmul(
                                pl[:, 0:w],
                                qT[:, n * 2 + hc, :],
                                kT_sb[:, hc, s0 + off:s0 + off + w],
                                start=(hc == 0), stop=(hc == 1),
                            )
                        if off == 0 and i >= NB:
                            nc.vector.tensor_add(pl[:, 0:P], pl[:, 0:P], mleft[:])
                        if off + w == width:
                            nc.vector.tensor_add(
                                pl[:, w - P:w], pl[:, w - P:w], mdiag[:]
                            )
                        if pi == 0:
                            acc_t = lacc
                        else:
                            acc_t = work.tile([P, 1], f32, name="lpart",
                                              tag="lpart")
                        nc.scalar.activation(
                            probs[:, off:off + w], pl[:, 0:w], ACTF.Exp,
                            bias=eshift_t[:], accum_out=acc_t[:],
                        )
                        if pi > 0:
                            nc.vector.tensor_add(lacc[:], lacc[:], acc_t[:])
                        off += w
                    rl = work.tile([P, 1], f32, name="rl", tag="rl")
                    nc.vector.reciprocal(rl[:], lacc[:])
                    nc.vector.tensor_scalar_mul(
                        probs[:, 0:width], probs[:, 0:width], rl[:]
                    )
                # ---- F: joint-head PV ----
                # transpose probs blocks for both heads (2 blocks x 2 heads
                # per PSUM bank); pT2[m][n] = probs_n[:, block g0+m].T
                pT2s = []
                for g0 in range(0, nblk, 2):
                    gw = min(2, nblk - g0)
                    ptp = ps_mix.tile([P, 512], f16, name="ptp", tag="mix")
                    for m in range(gw):
                        for n in range(2):
                            nc.tensor.transpose(
                                ptp[:, (m * 2 + n) * P:(m * 2 + n + 1) * P],
                                probs2[n][:, (g0 + m) * P:(g0 + m + 1) * P],
                                idh,
                            )
                    pT2 = work.tile([P, 2, 2, P], f16, name="pT2", tag="pT2",
                                    bufs=6)
                    nc.vector.tensor_copy(
                        pT2[:].rearrange("p a b q -> p (a b q)")[:, 0:gw * 2 * P],
                        ptp[:, 0:gw * 2 * P],
                    )
                    pT2s.append(pT2)
                # encT[h_in, (n, hc) blocks of 128 t-cols]
                for hc in range(2):
                    enc = ps_enc.tile([P, 2 * P], f32, name="enc", tag="enc")
                    for jj in range(nblk):
                        nc.tensor.matmul(
                            enc[:],
                            v_sb[:, j0 + jj, hc * P:(hc + 1) * P],
                            pT2s[jj // 2][:, jj % 2, :, :],
                            start=(jj == 0), stop=(jj == nblk - 1),
                        )
                    # enc cols = (n0 q | n1 q) -> encT blocks hc, 2+hc
                    encT4 = encT[:].rearrange("p (n c q) -> p n c q", n=2, c=2)
                    nc.vector.tensor_copy(
                        encT4[:, :, hc, :],
                        enc[:].rearrange("p (n q) -> p n q", n=2),
                    )

                # ---- H: out projection ----
                out_sb = work.tile([P, D], f32, name="out_sb", tag="out_sb")
                for dq in range(4):
                    po = ps_mix.tile([P, 512], f32, name="po", tag="mix")
                    for hh in range(4):
                        n, hc = divmod(hh, 2)
                        nc.tensor.matmul(
                            po[:],
                            encT[:, hh * P:(hh + 1) * P],
                            wo_sb[:, n, hc, dq * 512:(dq + 1) * 512],
                            start=(hh == 0), stop=(hh == 3),
                        )
                    nc.vector.tensor_copy(
                        out_sb[:, dq * 512:(dq + 1) * 512], po[:]
                    )
                nc.scalar.dma_start(out_d[i * P:(i + 1) * P, :], out_sb[:])

    nc.compile()
    return nc


def get_nc():
    if "nc" not in _CACHE:
        _CACHE["nc"] = _build_nc()
    return _CACHE["nc"]


def make_in_maps(x, segment_pos, q_w, kv_w, o_w, q_scale, k_scale):
    frac = 2.0 * np.arange(H // 2, dtype=np.float32) / np.float32(H)
    timescale = (ROPE_BASE ** frac).astype(np.float32)
    in_maps = []
    for c in range(NCORES):
        b, kv = divmod(c, 4)
        pos = segment_pos[b].astype(np.float32)
        sinusoid = pos[:, None] / timescale[None, :]
        in_maps.append({
            "identh": np.eye(P, dtype=np.float16),
            "x": np.ascontiguousarray(x[b].T, dtype=np.float16),
            "wq": np.ascontiguousarray(q_w[2 * kv:2 * kv + 2], dtype=np.float16),
            "wk": np.ascontiguousarray(kv_w[0, kv], dtype=np.float16),
            "wv": np.ascontiguousarray(kv_w[1, kv], dtype=np.float16),
            "wo": np.ascontiguousarray(o_w[2 * kv:2 * kv + 2], dtype=np.float16),
            "cos": np.cos(sinusoid).astype(np.float16),
            "sin": np.sin(sinusoid).astype(np.float16),
            "qs": (q_scale.astype(np.float32) * np.float32(SCALE)),
            "ks": k_scale.astype(np.float32),
        })
    return in_maps


def kernel(x, segment_pos, attn_mask, q_w, kv_w, o_w, q_scale, k_scale,
           _trace=False, _tmpdir=None):
    from concourse.bass_utils import run_bass_kernel_spmd

    nc = get_nc()
    in_maps = make_in_maps(x, segment_pos, q_w, kv_w, o_w, q_scale, k_scale)
    res = run_bass_kernel_spmd(
        nc, in_maps, core_ids=list(range(NCORES)),
        trace=_trace, tmpdir=_tmpdir,
    )
    out = np.zeros((B, T, D), dtype=np.float32)
    for c in range(NCORES):
        out[c // 4] += res.results[c]["out"]
    if _trace:
        _CACHE["last_result"] = res
    return out


# revision 20
# speedup vs baseline: 1.0195x; 1.0195x over previous
"""Trainium2 Bass kernel for GQA sliding-window attention.

Module: B=2, T=2048, D=2048, N=8 q-heads, K=4 kv-heads, H=256,
sliding window 1024, causal, RMSNorm on q/k, RoPE, softmax, out-proj.

Sharding (8 cores): batch (2) x kv-head (4).  Core c handles batch
c//4 and kv head c%4 (q heads 2*(c%4), 2*(c%4)+1).  Each core
produces a partial [T, D] output (its 2 heads' contribution); the
host sums the 4 partials per batch element.

All matmul operands are fp16 (PE runs 16-bit at full rate; fp32/fp32r
run at quarter rate on this hw).  Accumulation is always fp32 in PSUM,
softmax/rmsnorm statistics in fp32.  x is transposed on the host so
the contraction dim lands on partitions without on-device transposes.

Per-core device pipeline, per 128-row query tile i (16 tiles):
  A. DMA xT panel [128, ND, 128]; project q0|q1 and k|v (two N=512
     fp16 matmuls per d-chunk, accumulated over 16 d-chunks)
  B. RMSNorm q0/q1/k: ACT Square+accum_out (3x), one Sqrt, one DVE
     recip, fused scalar_tensor_tensor apply; v -> fp16 resident
  C. RoPE in fp16 (host cos/sin tables)
  D. PE-transpose q (4 tiles -> one PSUM bank -> 1 copy) and k
     (2 tiles -> 1 copy) -> qT (current), kT (resident [H, T])
  E. logits = qT.T @ kT over the sliding window (<=9 key blocks,
     fp16, <=512-col pieces), additive tri masks on edge blocks,
     ACT Exp(bias=-5) with accum_out -> probs(fp16) + row-sum l
  F. probs *= 1/l; PE-transpose probs blocks (batched 4/bank);
     PV: encT += v.T @ pT (fp16)
  G. encT -> SBUF (fp16)
  H. out[t-tile] = encT.T @ o_w (fp16, N=512), DMA PSUM -> DRAM.
"""

import numpy as np

B, T, D, N, K, H = 2, 2048, 2048, 8, 4, 256
P = 128
NT = T // P          # 16 query tiles
ND = D // P          # 16 contraction chunks
NB = 8               # max lookback blocks (window 1024 = 8 blocks)
WINDOW = 1024
SCALE = 0.0625
EPS = 1e-6
ROPE_BASE = 10000.0
NEG = -1e30
ESHIFT = 5.0         # exp(logit - 5): |logit|<=16 so max exp < 6e4 (fp16 safe)
NCORES = 8

_CACHE = {}


def _split_pieces(width):
    """Split width (multiple of 128) into <=512-col pieces."""
    n_p = -(-width // 512)
    pieces = []
    rem = width
    for _ in range(n_p - 1):
        w = max(128, (width // n_p // 128) * 128)
        pieces.append(w)
        rem -= w
    pieces.append(rem)
    return pieces


def _build_nc():
    import concourse.mybir as mybir
    import concourse.tile as tile
    from concourse import bacc
    from concourse.masks import make_causal_mask

    dt = mybir.dt
    f32, f16 = dt.float32, dt.float16
    MUL = mybir.AluOpType.mult
    ACTF = mybir.ActivationFunctionType

    nc = bacc.Bacc(
        "TRN2",
        target_bir_lowering=False,
        debug=False,
        enable_asserts=False,
        num_devices=NCORES,
    )

    xT_d = nc.dram_tensor("x", [D, T], f16, kind="ExternalInput").ap()
    wq_d = nc.dram_tensor("wq", [2, D, H], f16, kind="ExternalInput").ap()
    wk_d = nc.dram_tensor("wk", [D, H], f16, kind="ExternalInput").ap()
    wv_d = nc.dram_tensor("wv", [D, H], f16, kind="ExternalInput").ap()
    wo_d = nc.dram_tensor("wo", [2, H, D], f16, kind="ExternalInput").ap()
    idh_d = nc.dram_tensor("identh", [P, P], f16, kind="ExternalInput").ap()
    cos_d = nc.dram_tensor("cos", [T, H // 2], f16, kind="ExternalInput").ap()
    sin_d = nc.dram_tensor("sin", [T, H // 2], f16, kind="ExternalInput").ap()
    qs_d = nc.dram_tensor("qs", [H], f32, kind="ExternalInput").ap()
    ks_d = nc.dram_tensor("ks", [H], f32, kind="ExternalInput").ap()
    out_d = nc.dram_tensor("out", [T, D], f32, kind="ExternalOutput").ap()

    with tile.TileContext(nc) as tc:
        with (
            tc.tile_pool(name="consts", bufs=1) as consts,
            tc.tile_pool(name="ldx", bufs=5) as ldx,
            tc.tile_pool(name="work", bufs=3) as work,
            tc.tile_pool(name="ps_mix", bufs=3, space="PSUM") as ps_mix,
            tc.tile_pool(name="ps_proj", bufs=2, space="PSUM") as ps_proj,
            tc.tile_pool(name="ps_enc", bufs=1, space="PSUM") as ps_enc,
        ):
            # ---- constants / resident tensors ----
            ident_b = consts.tile([P, P], f16, name="ident_b")
            nc.scalar.dma_start(ident_b[:], idh_d[:])

            # mdiag: 0 where key<=query (lower tri incl diag), NEG above
            mdiag = consts.tile([P, P], f32, name="mdiag")
            make_causal_mask(nc, mdiag[:], mask_val=NEG)
            # mleft: NEG where key<=query, 0 above (strict upper allowed)
            mleft = consts.tile([P, P], f32, name="mleft")
            nc.gpsimd.memset(mleft[:], NEG)
            nc.gpsimd.affine_select(
                out=mleft[:],
                in_=mleft[:],
                compare_op=mybir.AluOpType.is_ge,
                fill=0.0,
                base=0,
                pattern=[[-1, P]],
                channel_multiplier=1,
            )

            eps_t = consts.tile([P, 1], f32, name="eps_t")
            nc.gpsimd.memset(eps_t[:], EPS)
            eshift_t = consts.tile([P, 1], f32, name="eshift_t")
            nc.gpsimd.memset(eshift_t[:], -ESHIFT)

            qs_t = consts.tile([P, H], f32, name="qs_t")
            nc.scalar.dma_start(qs_t[:], qs_d[None, :].broadcast_to([P, H]))
            ks_t = consts.tile([P, H], f32, name="ks_t")
            nc.scalar.dma_start(ks_t[:], ks_d[None, :].broadcast_to([P, H]))

            # weights: partition = d (within 128-chunk); per d-chunk the
            # two heads (or k|v) sit contiguously so one N=512 matmul
            # covers a full PSUM bank (one accumulation group per bank).
            wq_sb = consts.tile([P, ND, 2, H], f16, name="wq_sb")
            wkv_sb = consts.tile([P, ND, 2, H], f16, name="wkv_sb")
            wq_r = [wq_d[n].rearrange("(c p) h -> p c h", p=P) for n in range(2)]
            wk_r = wk_d.rearrange("(c p) h -> p c h", p=P)
            wv_r = wv_d.rearrange("(c p) h -> p c h", p=P)
            for dd in range(0, ND, 4):
                for n in range(2):
                    nc.scalar.dma_start(
                        wq_sb[:, dd:dd + 4, n, :], wq_r[n][:, dd:dd + 4, :]
                    )
                nc.sync.dma_start(
                    wkv_sb[:, dd:dd + 4, 0, :], wk_r[:, dd:dd + 4, :]
                )
                nc.sync.dma_start(
                    wkv_sb[:, dd:dd + 4, 1, :], wv_r[:, dd:dd + 4, :]
                )
            cos_sb = consts.tile([P, NT, H // 2], f16, name="cos_sb")
            nc.scalar.dma_start(cos_sb[:], cos_d.rearrange("(c p) f -> p c f", p=P))
            sin_sb = consts.tile([P, NT, H // 2], f16, name="sin_sb")
            nc.scalar.dma_start(sin_sb[:], sin_d.rearrange("(c p) f -> p c f", p=P))

            # wo: partition = h (within 128-chunk), [p, head, hc, D]
            wo_sb = consts.tile([P, 2, 2, D], f16, name="wo_sb")
            for n in range(2):
                nc.scalar.dma_start(
                    wo_sb[:, n, :, :],
                    wo_d[n].rearrange("(hc p) d -> p hc d", p=P),
                )

            # resident k/v for the whole sequence
            kT_sb = consts.tile([P, 2, T], f16, name="kT_sb")   # [h_in, hc, s]
            v_sb = consts.tile([P, NT, H], f16, name="v_sb")    # [s_in, sc, h]

            idh = ident_b[:]
            xT_r = xT_d.rearrange("(c p) t -> p c t", p=P)

            # PE warm-up: real matmuls (transpose-mode doesn't count for
            # HAM) on a memset tile keep the clock at K=8/8 through the
            # DMA-bound startup.
            wu_in = consts.tile([P, P], f16, name="wu_in")
            nc.gpsimd.memset(wu_in[:], 0.001)
            wu_ps = ps_mix.tile([P, 512], f32, name="wu_ps", tag="mix")
            for r in range(120):
                nc.tensor.matmul(wu_ps[:, 0:P], wu_in[:], wu_in[:],
                                 start=True, stop=True)

            for i in range(NT):
                # ---- A: projections ----
                xp = ldx.tile([P, ND, P], f16, name="xp", tag="xp")
                nc.sync.dma_start(xp[:], xT_r[:, :, i * P:(i + 1) * P])
                proj = ps_proj.tile([P, 4 * H], f32, name="proj", tag="proj")
                for d in range(ND):
                    st, sp = (d == 0), (d == ND - 1)
                    nc.tensor.matmul(
                        proj[:, 0:2 * H], xp[:, d, :],
                        wq_sb[:, d, :, :], start=st, stop=sp,
                    )
                    nc.tensor.matmul(
                        proj[:, 2 * H:4 * H], xp[:, d, :],
                        wkv_sb[:, d, :, :], start=st, stop=sp,
                    )

                # ---- B: RMSNorm q0,q1,k ; v -> fp16 resident ----
                qkhat = work.tile([P, 3 * H], f16, name="qkhat", tag="qkhat")
                sqs = work.tile([P, H], f32, name="sqs", tag="sqs")
                ssq3 = work.tile([P, 3], f32, name="ssq3", tag="ssq3")
                for j in range(3):
                    nc.scalar.activation(
                        sqs[:], proj[:, j * H:(j + 1) * H], ACTF.Square,
                        accum_out=ssq3[:, j:j + 1],
                    )
                rms3 = work.tile([P, 3], f32, name="rms3", tag="rms3")
                nc.scalar.activation(
                    rms3[:], ssq3[:], ACTF.Sqrt, bias=eps_t[:], scale=1.0 / H
                )
                rinv3 = work.tile([P, 3], f32, name="rinv3", tag="rinv3")
                nc.vector.reciprocal(rinv3[:], rms3[:])
                for j, sc in ((0, qs_t), (1, qs_t), (2, ks_t)):
                    nc.vector.scalar_tensor_tensor(
                        qkhat[:, j * H:(j + 1) * H],
                        proj[:, j * H:(j + 1) * H],
                        rinv3[:, j:j + 1], sc[:],
                        op0=MUL, op1=MUL,
                    )
                nc.vector.tensor_copy(v_sb[:, i, :], proj[:, 3 * H:4 * H])

                # ---- C: RoPE (fp16, q0|q1|k together) ----
                qkrot = work.tile([P, 3 * H], f16, name="qkrot", tag="qkrot")
                F = H // 2
                cos_i = cos_sb[:, i, :]
                sin_i = sin_sb[:, i, :]
                cb = cos_i[:, None, :].broadcast_to([P, 3, F])
                sb = sin_i[:, None, :].broadcast_to([P, 3, F])
                q4 = qkhat[:].rearrange("p (n two f) -> p n two f", n=3, two=2)
                qr4 = qkrot[:].rearrange("p (n two f) -> p n two f", n=3, two=2)
                t1 = work.tile([P, 3, F], f16, name="t1", tag="t1")
                t2 = work.tile([P, 3, F], f16, name="t2", tag="t2")
                nc.vector.tensor_mul(t1[:], q4[:, :, 0, :], cb)
                nc.vector.tensor_mul(t2[:], q4[:, :, 1, :], sb)
                nc.vector.tensor_sub(qr4[:, :, 0, :], t1[:], t2[:])
                nc.vector.tensor_mul(t1[:], q4[:, :, 1, :], cb)
                nc.vector.tensor_mul(t2[:], q4[:, :, 0, :], sb)
                nc.vector.tensor_add(qr4[:, :, 1, :], t1[:], t2[:])

                # ---- D: transposes (q: 4 into one bank, k: 2) ----
                tq = ps_mix.tile([P, 512], f16, name="tq", tag="mix")
                for m in range(4):
                    nc.tensor.transpose(
                        tq[:, m * P:(m + 1) * P],
                        qkrot[:, m * P:(m + 1) * P], idh,
                    )
                qT = work.tile([P, 4, P], f16, name="qT", tag="qT")
                nc.vector.tensor_copy(qT[:], tq[:])
                tk = ps_mix.tile([P, 512], f16, name="tk", tag="mix")
                for hc in range(2):
                    nc.tensor.transpose(
                        tk[:, hc * P:(hc + 1) * P],
                        qkrot[:, 2 * H + hc * P:2 * H + (hc + 1) * P], idh,
                    )
                nc.vector.tensor_copy(
                    kT_sb[:, :, i * P:(i + 1) * P],
                    tk[:, 0:2 * P].rearrange("p (hc q) -> p hc q", hc=2),
                )

                # ---- E: logits/softmax per head ----
                j0 = max(0, i - NB)
                nblk = i - j0 + 1
                width = nblk * P
                s0 = j0 * P
                pieces = _split_pieces(width)
                encT = work.tile([P, 2 * H], f16, name="encT", tag="encT")
                probs2 = []
                for n in range(2):
                    probs = work.tile([P, 9 * P], f16, name="probs", tag="probs")
                    probs2.append(probs)
                    lacc = work.tile([P, 1], f32, name="lacc", tag="lacc")
                    off = 0
                    for pi, w in enumerate(pieces):
                        pl = ps_mix.tile([P, 512], f32, name="pl", tag="mix")
                        for hc in range(2):
                            nc.tensor.matmul(
                                pl[:, 0:w],
                                qT[:, n * 2 + hc, :],
                                kT_sb[:, hc, s0 + off:s0 + off + w],
                                start=(hc == 0), stop=(hc == 1),
                            )
                        if off == 0 and i >= NB:
                            nc.vector.tensor_add(pl[:, 0:P], pl[:, 0:P], mleft[:])
                        if off + w == width:
                            nc.vector.tensor_add(
                                pl[:, w - P:w], pl[:, w - P:w], mdiag[:]
                            )
                        if pi == 0:
                            acc_t = lacc
                        else:
                            acc_t = work.tile([P, 1], f32, name="lpart",
                                              tag="lpart")
                        nc.scalar.activation(
                            probs[:, off:off + w], pl[:, 0:w], ACTF.Exp,
                            bias=eshift_t[:], accum_out=acc_t[:],
                        )
                        if pi > 0:
                            nc.vector.tensor_add(lacc[:], lacc[:], acc_t[:])
                        off += w
                    rl = work.tile([P, 1], f32, name="rl", tag="rl")
                    nc.vector.reciprocal(rl[:], lacc[:])
                    nc.vector.tensor_scalar_mul(
                        probs[:, 0:width], probs[:, 0:width], rl[:]
                    )
                # ---- F: joint-head PV ----
                # transpose probs blocks for both heads (2 blocks x 2 heads
                # per PSUM bank); pT2[m][n] = probs_n[:, block g0+m].T
                pT2s = []
                for g0 in range(0, nblk, 2):
                    gw = min(2, nblk - g0)
                    ptp = ps_mix.tile([P, 512], f16, name="ptp", tag="mix")
                    for m in range(gw):
                        for n in range(2):
                            nc.tensor.transpose(
                                ptp[:, (m * 2 + n) * P:(m * 2 + n + 1) * P],
                                probs2[n][:, (g0 + m) * P:(g0 + m + 1) * P],
                                idh,
                            )
                    pT2 = work.tile([P, 2, 2, P], f16, name="pT2", tag="pT2",
                                    bufs=6)
                    nc.vector.tensor_copy(
                        pT2[:].rearrange("p a b q -> p (a b q)")[:, 0:gw * 2 * P],
                        ptp[:, 0:gw * 2 * P],
                    )
                    pT2s.append(pT2)
                # encT[h_in, (n, hc) blocks of 128 t-cols]
                for hc in range(2):
                    enc = ps_enc.tile([P, 2 * P], f32, name="enc", tag="enc")
                    for jj in range(nblk):
                        nc.tensor.matmul(
                            enc[:],
                            v_sb[:, j0 + jj, hc * P:(hc + 1) * P],
                            pT2s[jj // 2][:, jj % 2, :, :],
                            start=(jj == 0), stop=(jj == nblk - 1),
                        )
                    # enc cols = (n0 q | n1 q) -> encT blocks hc, 2+hc
                    encT4 = encT[:].rearrange("p (n c q) -> p n c q", n=2, c=2)
                    nc.vector.tensor_copy(
                        encT4[:, :, hc, :],
                        enc[:].rearrange("p (n q) -> p n q", n=2),
                    )

                # ---- H: out projection ----
                out_sb = work.tile([P, D], f32, name="out_sb", tag="out_sb")
                for dq in range(4):
                    po = ps_mix.tile([P, 512], f32, name="po", tag="mix")
                    for hh in range(4):
                        n, hc = divmod(hh, 2)
                        nc.tensor.matmul(
                            po[:],
                            encT[:, hh * P:(hh + 1) * P],
                            wo_sb[:, n, hc, dq * 512:(dq + 1) * 512],
                            start=(hh == 0), stop=(hh == 3),
                        )
                    nc.vector.tensor_copy(
                        out_sb[:, dq * 512:(dq + 1) * 512], po[:]
                    )
                nc.scalar.dma_start(out_d[i * P:(i + 1) * P, :], out_sb[:])

    nc.compile()
    return nc


def get_nc():
    if "nc" not in _CACHE:
        _CACHE["nc"] = _build_nc()
    return _CACHE["nc"]


def make_in_maps(x, segment_pos, q_w, kv_w, o_w, q_scale, k_scale):
    frac = 2.0 * np.arange(H // 2, dtype=np.float32) / np.float32(H)
    timescale = (ROPE_BASE ** frac).astype(np.float32)
    in_maps = []
    for c in range(NCORES):
        b, kv = divmod(c, 4)
        pos = segment_pos[b].astype(np.float32)
        sinusoid = pos[:, None] / timescale[None, :]
        in_maps.append({
            "identh": np.eye(P, dtype=np.float16),
            "x": np.ascontiguousarray(x[b].T, dtype=np.float16),
            "wq": np.ascontiguousarray(q_w[2 * kv:2 * kv + 2], dtype=np.float16),
            "wk": np.ascontiguousarray(kv_w[0, kv], dtype=np.float16),
            "wv": np.ascontiguousarray(kv_w[1, kv], dtype=np.float16),
            "wo": np.ascontiguousarray(o_w[2 * kv:2 * kv + 2], dtype=np.float16),
            "cos": np.cos(sinusoid).astype(np.float16),
            "sin": np.sin(sinusoid).astype(np.float16),
            "qs": (q_scale.astype(np.float32) * np.float32(SCALE)),
            "ks": k_scale.astype(np.float32),
        })
    return in_maps


def kernel(x, segment_pos, attn_mask, q_w, kv_w, o_w, q_scale, k_scale,
           _trace=False, _tmpdir=None):
    from concourse.bass_utils import run_bass_kernel_spmd

    nc = get_nc()
    in_maps = make_in_maps(x, segment_pos, q_w, kv_w, o_w, q_scale, k_scale)
    res = run_bass_kernel_spmd(
        nc, in_maps, core_ids=list(range(NCORES)),
        trace=_trace, tmpdir=_tmpdir,
    )
    out = np.zeros((B, T, D), dtype=np.float32)
    for c in range(NCORES):
        out[c // 4] += res.results[c]["out"]
    if _trace:
        _CACHE["last_result"] = res
    return out


# revision 21
# speedup vs baseline: 1.0232x; 1.0037x over previous
"""Trainium2 Bass kernel for GQA sliding-window attention.

Module: B=2, T=2048, D=2048, N=8 q-heads, K=4 kv-heads, H=256,
sliding window 1024, causal, RMSNorm on q/k, RoPE, softmax, out-proj.

Sharding (8 cores): batch (2) x kv-head (4).  Core c handles batch
c//4 and kv head c%4 (q heads 2*(c%4), 2*(c%4)+1).  Each core
produces a partial [T, D] output (its 2 heads' contribution); the
host sums the 4 partials per batch element.

All matmul operands are fp16 (PE runs 16-bit at full rate; fp32/fp32r
run at quarter rate on this hw).  Accumulation is always fp32 in PSUM,
softmax/rmsnorm statistics in fp32.  x is transposed on the host so
the contraction dim lands on partitions without on-device transposes.

Per-core device pipeline, per 128-row query tile i (16 tiles):
  A. DMA xT panel [128, ND, 128]; project q0|q1 and k|v (two N=512
     fp16 matmuls per d-chunk, accumulated over 16 d-chunks)
  B. RMSNorm q0/q1/k: ACT Square+accum_out (3x), one Sqrt, one DVE
     recip, fused scalar_tensor_tensor apply; v -> fp16 resident
  C. RoPE in fp16 (host cos/sin tables)
  D. PE-transpose q (4 tiles -> one PSUM bank -> 1 copy) and k
     (2 tiles -> 1 copy) -> qT (current), kT (resident [H, T])
  E. logits = qT.T @ kT over the sliding window (<=9 key blocks,
     fp16, <=512-col pieces), additive tri masks on edge blocks,
     ACT Exp(bias=-5) with accum_out -> probs(fp16) + row-sum l
  F. probs *= 1/l; PE-transpose probs blocks (batched 4/bank);
     PV: encT += v.T @ pT (fp16)
  G. encT -> SBUF (fp16)
  H. out[t-tile] = encT.T @ o_w (fp16, N=512), DMA PSUM -> DRAM.
"""

import numpy as np

B, T, D, N, K, H = 2, 2048, 2048, 8, 4, 256
P = 128
NT = T // P          # 16 query tiles
ND = D // P          # 16 contraction chunks
NB = 8               # max lookback blocks (window 1024 = 8 blocks)
WINDOW = 1024
SCALE = 0.0625
EPS = 1e-6
ROPE_BASE = 10000.0
NEG = -1e30
ESHIFT = 5.0         # exp(logit - 5): |logit|<=16 so max exp < 6e4 (fp16 safe)
NCORES = 8

_CACHE = {}


def _split_pieces(width):
    """Split width (multiple of 128) into <=512-col pieces."""
    n_p = -(-width // 512)
    pieces = []
    rem = width
    for _ in range(n_p - 1):
        w = max(128, (width // n_p // 128) * 128)
        pieces.append(w)
        rem -= w
    pieces.append(rem)
    return pieces


def _build_nc():
    import concourse.mybir as mybir
    import concourse.tile as tile
    from concourse import bacc
    from concourse.masks import make_causal_mask

    dt = mybir.dt
    f32, f16 = dt.float32, dt.float16
    MUL = mybir.AluOpType.mult
    ACTF = mybir.ActivationFunctionType

    nc = bacc.Bacc(
        "TRN2",
        target_bir_lowering=False,
        debug=False,
        enable_asserts=False,
        num_devices=NCORES,
    )

    xT_d = nc.dram_tensor("x", [D, T], f16, kind="ExternalInput").ap()
    wq_d = nc.dram_tensor("wq", [2, D, H], f16, kind="ExternalInput").ap()
    wk_d = nc.dram_tensor("wk", [D, H], f16, kind="ExternalInput").ap()
    wv_d = nc.dram_tensor("wv", [D, H], f16, kind="ExternalInput").ap()
    wo_d = nc.dram_tensor("wo", [2, H, D], f16, kind="ExternalInput").ap()
    idh_d = nc.dram_tensor("identh", [P, P], f16, kind="ExternalInput").ap()
    cos_d = nc.dram_tensor("cos", [T, H // 2], f16, kind="ExternalInput").ap()
    sin_d = nc.dram_tensor("sin", [T, H // 2], f16, kind="ExternalInput").ap()
    qs_d = nc.dram_tensor("qs", [H], f32, kind="ExternalInput").ap()
    ks_d = nc.dram_tensor("ks", [H], f32, kind="ExternalInput").ap()
    out_d = nc.dram_tensor("out", [T, D], f32, kind="ExternalOutput").ap()

    with tile.TileContext(nc) as tc:
        with (
            tc.tile_pool(name="consts", bufs=1) as consts,
            tc.tile_pool(name="ldx", bufs=5) as ldx,
            tc.tile_pool(name="work", bufs=3) as work,
            tc.tile_pool(name="ps_mix", bufs=3, space="PSUM") as ps_mix,
            tc.tile_pool(name="ps_proj", bufs=2, space="PSUM") as ps_proj,
            tc.tile_pool(name="ps_enc", bufs=1, space="PSUM") as ps_enc,
        ):
            # ---- constants / resident tensors ----
            ident_b = consts.tile([P, P], f16, name="ident_b")
            nc.scalar.dma_start(ident_b[:], idh_d[:])

            # mdiag: 0 where key<=query (lower tri incl diag), NEG above
            mdiag = consts.tile([P, P], f32, name="mdiag")
            make_causal_mask(nc, mdiag[:], mask_val=NEG)
            # mleft: NEG where key<=query, 0 above (strict upper allowed)
            mleft = consts.tile([P, P], f32, name="mleft")
            nc.gpsimd.memset(mleft[:], NEG)
            nc.gpsimd.affine_select(
                out=mleft[:],
                in_=mleft[:],
                compare_op=mybir.AluOpType.is_ge,
                fill=0.0,
                base=0,
                pattern=[[-1, P]],
                channel_multiplier=1,
            )

            eps_t = consts.tile([P, 1], f32, name="eps_t")
            nc.gpsimd.memset(eps_t[:], EPS)
            eshift_t = consts.tile([P, 1], f32, name="eshift_t")
            nc.gpsimd.memset(eshift_t[:], -ESHIFT)

            qs_t = consts.tile([P, H], f32, name="qs_t")
            nc.scalar.dma_start(qs_t[:], qs_d[None, :].broadcast_to([P, H]))
            ks_t = consts.tile([P, H], f32, name="ks_t")
            nc.scalar.dma_start(ks_t[:], ks_d[None, :].broadcast_to([P, H]))

            # weights: partition = d (within 128-chunk); per d-chunk the
            # two heads (or k|v) sit contiguously so one N=512 matmul
            # covers a full PSUM bank (one accumulation group per bank).
            wq_sb = consts.tile([P, ND, 2, H], f16, name="wq_sb")
            wkv_sb = consts.tile([P, ND, 2, H], f16, name="wkv_sb")
            wq_r = [wq_d[n].rearrange("(c p) h -> p c h", p=P) for n in range(2)]
            wk_r = wk_d.rearrange("(c p) h -> p c h", p=P)
            wv_r = wv_d.rearrange("(c p) h -> p c h", p=P)
            for dd in range(0, ND, 4):
                for n in range(2):
                    nc.scalar.dma_start(
                        wq_sb[:, dd:dd + 4, n, :], wq_r[n][:, dd:dd + 4, :]
                    )
                nc.sync.dma_start(
                    wkv_sb[:, dd:dd + 4, 0, :], wk_r[:, dd:dd + 4, :]
                )
                nc.sync.dma_start(
                    wkv_sb[:, dd:dd + 4, 1, :], wv_r[:, dd:dd + 4, :]
                )
            cos_sb = consts.tile([P, NT, H // 2], f16, name="cos_sb")
            nc.scalar.dma_start(cos_sb[:], cos_d.rearrange("(c p) f -> p c f", p=P))
            sin_sb = consts.tile([P, NT, H // 2], f16, name="sin_sb")
            nc.scalar.dma_start(sin_sb[:], sin_d.rearrange("(c p) f -> p c f", p=P))

            # wo: partition = h (within 128-chunk), [p, head, hc, D]
            wo_sb = consts.tile([P, 2, 2, D], f16, name="wo_sb")
            for n in range(2):
                nc.scalar.dma_start(
                    wo_sb[:, n, :, :],
                    wo_d[n].rearrange("(hc p) d -> p hc d", p=P),
                )

            # resident k/v for the whole sequence
            kT_sb = consts.tile([P, 2, T], f16, name="kT_sb")   # [h_in, hc, s]
            v_sb = consts.tile([P, NT, H], f16, name="v_sb")    # [s_in, sc, h]

            idh = ident_b[:]
            xT_r = xT_d.rearrange("(c p) t -> p c t", p=P)

            # PE warm-up: real matmuls (transpose-mode doesn't count for
            # HAM) on a memset tile keep the clock at K=8/8 through the
            # DMA-bound startup.
            wu_in = consts.tile([P, P], f16, name="wu_in")
            nc.gpsimd.memset(wu_in[:], 0.001)
            wu_ps = ps_mix.tile([P, 512], f32, name="wu_ps", tag="mix")
            for r in range(40):
                nc.tensor.matmul(wu_ps[:, 0:P], wu_in[:], wu_in[:],
                                 start=True, stop=True)

            for i in range(NT):
                # ---- A: projections ----
                xp = ldx.tile([P, ND, P], f16, name="xp", tag="xp")
                nc.sync.dma_start(xp[:], xT_r[:, :, i * P:(i + 1) * P])
                proj = ps_proj.tile([P, 4 * H], f32, name="proj", tag="proj")
                # k|v first: attention depends (via kT) only on this half
                for d in range(ND):
                    nc.tensor.matmul(
                        proj[:, 2 * H:4 * H], xp[:, d, :],
                        wkv_sb[:, d, :, :], start=(d == 0), stop=(d == ND - 1),
                    )
                for d in range(ND):
                    nc.tensor.matmul(
                        proj[:, 0:2 * H], xp[:, d, :],
                        wq_sb[:, d, :, :], start=(d == 0), stop=(d == ND - 1),
                    )

                # ---- B: RMSNorm q0,q1,k ; v -> fp16 resident ----
                qkhat = work.tile([P, 3 * H], f16, name="qkhat", tag="qkhat")
                sqs = work.tile([P, H], f32, name="sqs", tag="sqs")
                ssq3 = work.tile([P, 3], f32, name="ssq3", tag="ssq3")
                for j in range(3):
                    nc.scalar.activation(
                        sqs[:], proj[:, j * H:(j + 1) * H], ACTF.Square,
                        accum_out=ssq3[:, j:j + 1],
                    )
                rms3 = work.tile([P, 3], f32, name="rms3", tag="rms3")
                nc.scalar.activation(
                    rms3[:], ssq3[:], ACTF.Sqrt, bias=eps_t[:], scale=1.0 / H
                )
                rinv3 = work.tile([P, 3], f32, name="rinv3", tag="rinv3")
                nc.vector.reciprocal(rinv3[:], rms3[:])
                for j, sc in ((0, qs_t), (1, qs_t), (2, ks_t)):
                    nc.vector.scalar_tensor_tensor(
                        qkhat[:, j * H:(j + 1) * H],
                        proj[:, j * H:(j + 1) * H],
                        rinv3[:, j:j + 1], sc[:],
                        op0=MUL, op1=MUL,
                    )
                nc.vector.tensor_copy(v_sb[:, i, :], proj[:, 3 * H:4 * H])

                # ---- C: RoPE (fp16, q0|q1|k together) ----
                qkrot = work.tile([P, 3 * H], f16, name="qkrot", tag="qkrot")
                F = H // 2
                cos_i = cos_sb[:, i, :]
                sin_i = sin_sb[:, i, :]
                cb = cos_i[:, None, :].broadcast_to([P, 3, F])
                sb = sin_i[:, None, :].broadcast_to([P, 3, F])
                q4 = qkhat[:].rearrange("p (n two f) -> p n two f", n=3, two=2)
                qr4 = qkrot[:].rearrange("p (n two f) -> p n two f", n=3, two=2)
                t1 = work.tile([P, 3, F], f16, name="t1", tag="t1")
                t2 = work.tile([P, 3, F], f16, name="t2", tag="t2")
                nc.vector.tensor_mul(t1[:], q4[:, :, 0, :], cb)
                nc.vector.tensor_mul(t2[:], q4[:, :, 1, :], sb)
                nc.vector.tensor_sub(qr4[:, :, 0, :], t1[:], t2[:])
                nc.vector.tensor_mul(t1[:], q4[:, :, 1, :], cb)
                nc.vector.tensor_mul(t2[:], q4[:, :, 0, :], sb)
                nc.vector.tensor_add(qr4[:, :, 1, :], t1[:], t2[:])

                # ---- D: transposes (q: 4 into one bank, k: 2) ----
                tq = ps_mix.tile([P, 512], f16, name="tq", tag="mix")
                for m in range(4):
                    nc.tensor.transpose(
                        tq[:, m * P:(m + 1) * P],
                        qkrot[:, m * P:(m + 1) * P], idh,
                    )
                qT = work.tile([P, 4, P], f16, name="qT", tag="qT")
                nc.vector.tensor_copy(qT[:], tq[:])
                tk = ps_mix.tile([P, 512], f16, name="tk", tag="mix")
                for hc in range(2):
                    nc.tensor.transpose(
                        tk[:, hc * P:(hc + 1) * P],
                        qkrot[:, 2 * H + hc * P:2 * H + (hc + 1) * P], idh,
                    )
                nc.vector.tensor_copy(
                    kT_sb[:, :, i * P:(i + 1) * P],
                    tk[:, 0:2 * P].rearrange("p (hc q) -> p hc q", hc=2),
                )

                # ---- E: logits/softmax per head ----
                j0 = max(0, i - NB)
                nblk = i - j0 + 1
                width = nblk * P
                s0 = j0 * P
                pieces = _split_pieces(width)
                encT = work.tile([P, 2 * H], f16, name="encT", tag="encT")
                probs2 = []
                for n in range(2):
                    probs = work.tile([P, 9 * P], f16, name="probs", tag="probs")
                    probs2.append(probs)
                    lacc = work.tile([P, 1], f32, name="lacc", tag="lacc")
                    off = 0
                    for pi, w in enumerate(pieces):
                        pl = ps_mix.tile([P, 512], f32, name="pl", tag="mix")
                        for hc in range(2):
                            nc.tensor.matmul(
                                pl[:, 0:w],
                                qT[:, n * 2 + hc, :],
                                kT_sb[:, hc, s0 + off:s0 + off + w],
                                start=(hc == 0), stop=(hc == 1),
                            )
                        if off == 0 and i >= NB:
                            nc.vector.tensor_add(pl[:, 0:P], pl[:, 0:P], mleft[:])
                        if off + w == width:
                            nc.vector.tensor_add(
                                pl[:, w - P:w], pl[:, w - P:w], mdiag[:]
                            )
                        if pi == 0:
                            acc_t = lacc
                        else:
                            acc_t = work.tile([P, 1], f32, name="lpart",
                                              tag="lpart")
                        nc.scalar.activation(
                            probs[:, off:off + w], pl[:, 0:w], ACTF.Exp,
                            bias=eshift_t[:], accum_out=acc_t[:],
                        )
                        if pi > 0:
                            nc.vector.tensor_add(lacc[:], lacc[:], acc_t[:])
                        off += w
                    rl = work.tile([P, 1], f32, name="rl", tag="rl")
                    nc.vector.reciprocal(rl[:], lacc[:])
                    nc.vector.tensor_scalar_mul(
                        probs[:, 0:width], probs[:, 0:width], rl[:]
                    )
                # ---- F: joint-head PV ----
                # transpose probs blocks for both heads (2 blocks x 2 heads
                # per PSUM bank); pT2[m][n] = probs_n[:, block g0+m].T
                pT2s = []
                for g0 in range(0, nblk, 2):
                    gw = min(2, nblk - g0)
                    ptp = ps_mix.tile([P, 512], f16, name="ptp", tag="mix")
                    for m in range(gw):
                        for n in range(2):
                            nc.tensor.transpose(
                                ptp[:, (m * 2 + n) * P:(m * 2 + n + 1) * P],
                                probs2[n][:, (g0 + m) * P:(g0 + m + 1) * P],
                                idh,
                            )
                    pT2 = work.tile([P, 2, 2, P], f16, name="pT2", tag="pT2",
                                    bufs=6)
                    nc.vector.tensor_copy(
                        pT2[:].rearrange("p a b q -> p (a b q)")[:, 0:gw * 2 * P],
                        ptp[:, 0:gw * 2 * P],
                    )
                    pT2s.append(pT2)
                # encT[h_in, (n, hc) blocks of 128 t-cols]
                for hc in range(2):
                    enc = ps_enc.tile([P, 2 * P], f32, name="enc", tag="enc")
                    for jj in range(nblk):
                        nc.tensor.matmul(
                            enc[:],
                            v_sb[:, j0 + jj, hc * P:(hc + 1) * P],
                            pT2s[jj // 2][:, jj % 2, :, :],
                            start=(jj == 0), stop=(jj == nblk - 1),
                        )
                    # enc cols = (n0 q | n1 q) -> encT blocks hc, 2+hc
                    encT4 = encT[:].rearrange("p (n c q) -> p n c q", n=2, c=2)
                    nc.vector.tensor_copy(
                        encT4[:, :, hc, :],
                        enc[:].rearrange("p (n q) -> p n q", n=2),
                    )

                # ---- H: out projection ----
                out_sb = work.tile([P, D], f32, name="out_sb", tag="out_sb")
                for dq in range(4):
                    po = ps_mix.tile([P, 512], f32, name="po", tag="mix")
                    for hh in range(4):
                        n, hc = divmod(hh, 2)
                        nc.tensor.matmul(
                            po[:],
                            encT[:, hh * P:(hh + 1) * P],
                            wo_sb[:, n, hc, dq * 512:(dq + 1) * 512],
                            start=(hh == 0), stop=(hh == 3),
                        )
                    nc.vector.tensor_copy(
                        out_sb[:, dq * 512:(dq + 1) * 512], po[:]
                    )
                nc.scalar.dma_start(out_d[i * P:(i + 1) * P, :], out_sb[:])

    nc.compile()
    return nc


def get_nc():
    if "nc" not in _CACHE:
        _CACHE["nc"] = _build_nc()
    return _CACHE["nc"]


def make_in_maps(x, segment_pos, q_w, kv_w, o_w, q_scale, k_scale):
    frac = 2.0 * np.arange(H // 2, dtype=np.float32) / np.float32(H)
    timescale = (ROPE_BASE ** frac).astype(np.float32)
    in_maps = []
    for c in range(NCORES):
        b, kv = divmod(c, 4)
        pos = segment_pos[b].astype(np.float32)
        sinusoid = pos[:, None] / timescale[None, :]
        in_maps.append({
            "identh": np.eye(P, dtype=np.float16),
            "x": np.ascontiguousarray(x[b].T, dtype=np.float16),
            "wq": np.ascontiguousarray(q_w[2 * kv:2 * kv + 2], dtype=np.float16),
            "wk": np.ascontiguousarray(kv_w[0, kv], dtype=np.float16),
            "wv": np.ascontiguousarray(kv_w[1, kv], dtype=np.float16),
            "wo": np.ascontiguousarray(o_w[2 * kv:2 * kv + 2], dtype=np.float16),
            "cos": np.cos(sinusoid).astype(np.float16),
            "sin": np.sin(sinusoid).astype(np.float16),
            "qs": (q_scale.astype(np.float32) * np.float32(SCALE)),
            "ks": k_scale.astype(np.float32),
        })
    return in_maps


def kernel(x, segment_pos, attn_mask, q_w, kv_w, o_w, q_scale, k_scale,
           _trace=False, _tmpdir=None):
    from concourse.bass_utils import run_bass_kernel_spmd

    nc = get_nc()
    in_maps = make_in_maps(x, segment_pos, q_w, kv_w, o_w, q_scale, k_scale)
    res = run_bass_kernel_spmd(
        nc, in_maps, core_ids=list(range(NCORES)),
        trace=_trace, tmpdir=_tmpdir,
    )
    out = np.zeros((B, T, D), dtype=np.float32)
    for c in range(NCORES):
        out[c // 4] += res.results[c]["out"]
    if _trace:
        _CACHE["last_result"] = res
    return out


# revision 22
# speedup vs baseline: 1.0299x; 1.0065x over previous
"""Trainium2 Bass kernel for GQA sliding-window attention.

Module: B=2, T=2048, D=2048, N=8 q-heads, K=4 kv-heads, H=256,
sliding window 1024, causal, RMSNorm on q/k, RoPE, softmax, out-proj.

Sharding (8 cores): batch (2) x kv-head (4).  Core c handles batch
c//4 and kv head c%4 (q heads 2*(c%4), 2*(c%4)+1).  Each core
produces a partial [T, D] output (its 2 heads' contribution); the
host sums the 4 partials per batch element.

All matmul operands are fp16 (PE runs 16-bit at full rate; fp32/fp32r
run at quarter rate on this hw).  Accumulation is always fp32 in PSUM,
softmax/rmsnorm statistics in fp32.  x is transposed on the host so
the contraction dim lands on partitions without on-device transposes.

Per-core device pipeline, per 128-row query tile i (16 tiles):
  A. DMA xT panel [128, ND, 128]; project q0|q1 and k|v (two N=512
     fp16 matmuls per d-chunk, accumulated over 16 d-chunks)
  B. RMSNorm q0/q1/k: ACT Square+accum_out (3x), one Sqrt, one DVE
     recip, fused scalar_tensor_tensor apply; v -> fp16 resident
  C. RoPE in fp16 (host cos/sin tables)
  D. PE-transpose q (4 tiles -> one PSUM bank -> 1 copy) and k
     (2 tiles -> 1 copy) -> qT (current), kT (resident [H, T])
  E. logits = qT.T @ kT over the sliding window (<=9 key blocks,
     fp16, <=512-col pieces), additive tri masks on edge blocks,
     ACT Exp(bias=-5) with accum_out -> probs(fp16) + row-sum l
  F. probs *= 1/l; PE-transpose probs blocks (batched 4/bank);
     PV: encT += v.T @ pT (fp16)
  G. encT -> SBUF (fp16)
  H. out[t-tile] = encT.T @ o_w (fp16, N=512), DMA PSUM -> DRAM.
"""

import numpy as np

B, T, D, N, K, H = 2, 2048, 2048, 8, 4, 256
P = 128
NT = T // P          # 16 query tiles
ND = D // P          # 16 contraction chunks
NB = 8               # max lookback blocks (window 1024 = 8 blocks)
WINDOW = 1024
SCALE = 0.0625
EPS = 1e-6
ROPE_BASE = 10000.0
NEG = -1e30
ESHIFT = 5.0         # exp(logit - 5): |logit|<=16 so max exp < 6e4 (fp16 safe)
NCORES = 8

_CACHE = {}


def _split_pieces(width):
    """Split width (multiple of 128) into <=512-col pieces."""
    n_p = -(-width // 512)
    pieces = []
    rem = width
    for _ in range(n_p - 1):
        w = max(128, (width // n_p // 128) * 128)
        pieces.append(w)
        rem -= w
    pieces.append(rem)
    return pieces


def _build_nc():
    import concourse.mybir as mybir
    import concourse.tile as tile
    from concourse import bacc
    from concourse.masks import make_causal_mask

    dt = mybir.dt
    f32, f16 = dt.float32, dt.float16
    MUL = mybir.AluOpType.mult
    ACTF = mybir.ActivationFunctionType

    nc = bacc.Bacc(
        "TRN2",
        target_bir_lowering=False,
        debug=False,
        enable_asserts=False,
        num_devices=NCORES,
    )

    xT_d = nc.dram_tensor("x", [D, T], f16, kind="ExternalInput").ap()
    wq_d = nc.dram_tensor("wq", [2, D, H], f16, kind="ExternalInput").ap()
    wk_d = nc.dram_tensor("wk", [D, H], f16, kind="ExternalInput").ap()
    wv_d = nc.dram_tensor("wv", [D, H], f16, kind="ExternalInput").ap()
    wo_d = nc.dram_tensor("wo", [2, H, D], f16, kind="ExternalInput").ap()
    idh_d = nc.dram_tensor("identh", [P, P], f16, kind="ExternalInput").ap()
    cos_d = nc.dram_tensor("cos", [T, H // 2], f16, kind="ExternalInput").ap()
    sin_d = nc.dram_tensor("sin", [T, H // 2], f16, kind="ExternalInput").ap()
    qs_d = nc.dram_tensor("qs", [H], f32, kind="ExternalInput").ap()
    ks_d = nc.dram_tensor("ks", [H], f32, kind="ExternalInput").ap()
    out_d = nc.dram_tensor("out", [T, D], f32, kind="ExternalOutput").ap()

    with tile.TileContext(nc) as tc:
        with (
            tc.tile_pool(name="consts", bufs=1) as consts,
            tc.tile_pool(name="ldx", bufs=5) as ldx,
            tc.tile_pool(name="work", bufs=4) as work,
            tc.tile_pool(name="ps_mix", bufs=3, space="PSUM") as ps_mix,
            tc.tile_pool(name="ps_proj", bufs=2, space="PSUM") as ps_proj,
            tc.tile_pool(name="ps_enc", bufs=1, space="PSUM") as ps_enc,
        ):
            # ---- constants / resident tensors ----
            ident_b = consts.tile([P, P], f16, name="ident_b")
            nc.scalar.dma_start(ident_b[:], idh_d[:])

            # mdiag: 0 where key<=query (lower tri incl diag), NEG above
            mdiag = consts.tile([P, P], f32, name="mdiag")
            make_causal_mask(nc, mdiag[:], mask_val=NEG)
            # mleft: NEG where key<=query, 0 above (strict upper allowed)
            mleft = consts.tile([P, P], f32, name="mleft")
            nc.gpsimd.memset(mleft[:], NEG)
            nc.gpsimd.affine_select(
                out=mleft[:],
                in_=mleft[:],
                compare_op=mybir.AluOpType.is_ge,
                fill=0.0,
                base=0,
                pattern=[[-1, P]],
                channel_multiplier=1,
            )

            eps_t = consts.tile([P, 1], f32, name="eps_t")
            nc.gpsimd.memset(eps_t[:], EPS)
            eshift_t = consts.tile([P, 1], f32, name="eshift_t")
            nc.gpsimd.memset(eshift_t[:], -ESHIFT)

            qs_t = consts.tile([P, H], f32, name="qs_t")
            nc.scalar.dma_start(qs_t[:], qs_d[None, :].broadcast_to([P, H]))
            ks_t = consts.tile([P, H], f32, name="ks_t")
            nc.scalar.dma_start(ks_t[:], ks_d[None, :].broadcast_to([P, H]))

            # weights: partition = d (within 128-chunk); per d-chunk the
            # two heads (or k|v) sit contiguously so one N=512 matmul
            # covers a full PSUM bank (one accumulation group per bank).
            wq_sb = consts.tile([P, ND, 2, H], f16, name="wq_sb")
            wkv_sb = consts.tile([P, ND, 2, H], f16, name="wkv_sb")
            wq_r = [wq_d[n].rearrange("(c p) h -> p c h", p=P) for n in range(2)]
            wk_r = wk_d.rearrange("(c p) h -> p c h", p=P)
            wv_r = wv_d.rearrange("(c p) h -> p c h", p=P)
            for dd in range(0, ND, 4):
                for n in range(2):
                    nc.scalar.dma_start(
                        wq_sb[:, dd:dd + 4, n, :], wq_r[n][:, dd:dd + 4, :]
                    )
                nc.sync.dma_start(
                    wkv_sb[:, dd:dd + 4, 0, :], wk_r[:, dd:dd + 4, :]
                )
                nc.sync.dma_start(
                    wkv_sb[:, dd:dd + 4, 1, :], wv_r[:, dd:dd + 4, :]
                )
            cos_sb = consts.tile([P, NT, H // 2], f16, name="cos_sb")
            nc.scalar.dma_start(cos_sb[:], cos_d.rearrange("(c p) f -> p c f", p=P))
            sin_sb = consts.tile([P, NT, H // 2], f16, name="sin_sb")
            nc.scalar.dma_start(sin_sb[:], sin_d.rearrange("(c p) f -> p c f", p=P))

            # wo: partition = h (within 128-chunk), [p, head, hc, D]
            wo_sb = consts.tile([P, 2, 2, D], f16, name="wo_sb")
            for n in range(2):
                nc.scalar.dma_start(
                    wo_sb[:, n, :, :],
                    wo_d[n].rearrange("(hc p) d -> p hc d", p=P),
                )

            # resident k/v for the whole sequence
            kT_sb = consts.tile([P, 2, T], f16, name="kT_sb")   # [h_in, hc, s]
            v_sb = consts.tile([P, NT, H], f16, name="v_sb")    # [s_in, sc, h]

            idh = ident_b[:]
            xT_r = xT_d.rearrange("(c p) t -> p c t", p=P)


            for i in range(NT):
                # ---- A: projections ----
                xp = ldx.tile([P, ND, P], f16, name="xp", tag="xp")
                nc.sync.dma_start(xp[:], xT_r[:, :, i * P:(i + 1) * P])
                proj = ps_proj.tile([P, 4 * H], f32, name="proj", tag="proj")
                # k|v first: attention depends (via kT) only on this half
                for d in range(ND):
                    nc.tensor.matmul(
                        proj[:, 2 * H:4 * H], xp[:, d, :],
                        wkv_sb[:, d, :, :], start=(d == 0), stop=(d == ND - 1),
                    )
                for d in range(ND):
                    nc.tensor.matmul(
                        proj[:, 0:2 * H], xp[:, d, :],
                        wq_sb[:, d, :, :], start=(d == 0), stop=(d == ND - 1),
                    )

                # ---- B: RMSNorm q0,q1,k ; v -> fp16 resident ----
                qkhat = work.tile([P, 3 * H], f16, name="qkhat", tag="qkhat")
                sqs = work.tile([P, H], f32, name="sqs", tag="sqs")
                ssq3 = work.tile([P, 3], f32, name="ssq3", tag="ssq3")
                for j in range(3):
                    nc.scalar.activation(
                        sqs[:], proj[:, j * H:(j + 1) * H], ACTF.Square,
                        accum_out=ssq3[:, j:j + 1],
                    )
                rms3 = work.tile([P, 3], f32, name="rms3", tag="rms3")
                nc.scalar.activation(
                    rms3[:], ssq3[:], ACTF.Sqrt, bias=eps_t[:], scale=1.0 / H
                )
                rinv3 = work.tile([P, 3], f32, name="rinv3", tag="rinv3")
                nc.vector.reciprocal(rinv3[:], rms3[:])
                for j, sc in ((0, qs_t), (1, qs_t), (2, ks_t)):
                    nc.vector.scalar_tensor_tensor(
                        qkhat[:, j * H:(j + 1) * H],
                        proj[:, j * H:(j + 1) * H],
                        rinv3[:, j:j + 1], sc[:],
                        op0=MUL, op1=MUL,
                    )
                nc.vector.tensor_copy(v_sb[:, i, :], proj[:, 3 * H:4 * H])

                # ---- C: RoPE (fp16, q0|q1|k together) ----
                qkrot = work.tile([P, 3 * H], f16, name="qkrot", tag="qkrot")
                F = H // 2
                cos_i = cos_sb[:, i, :]
                sin_i = sin_sb[:, i, :]
                cb = cos_i[:, None, :].broadcast_to([P, 3, F])
                sb = sin_i[:, None, :].broadcast_to([P, 3, F])
                q4 = qkhat[:].rearrange("p (n two f) -> p n two f", n=3, two=2)
                qr4 = qkrot[:].rearrange("p (n two f) -> p n two f", n=3, two=2)
                t1 = work.tile([P, 3, F], f16, name="t1", tag="t1")
                t2 = work.tile([P, 3, F], f16, name="t2", tag="t2")
                nc.vector.tensor_mul(t1[:], q4[:, :, 0, :], cb)
                nc.vector.tensor_mul(t2[:], q4[:, :, 1, :], sb)
                nc.vector.tensor_sub(qr4[:, :, 0, :], t1[:], t2[:])
                nc.vector.tensor_mul(t1[:], q4[:, :, 1, :], cb)
                nc.vector.tensor_mul(t2[:], q4[:, :, 0, :], sb)
                nc.vector.tensor_add(qr4[:, :, 1, :], t1[:], t2[:])

                # ---- D: transposes (q: 4 into one bank, k: 2) ----
                tq = ps_mix.tile([P, 512], f16, name="tq", tag="mix")
                for m in range(4):
                    nc.tensor.transpose(
                        tq[:, m * P:(m + 1) * P],
                        qkrot[:, m * P:(m + 1) * P], idh,
                    )
                qT = work.tile([P, 4, P], f16, name="qT", tag="qT")
                nc.vector.tensor_copy(qT[:], tq[:])
                tk = ps_mix.tile([P, 512], f16, name="tk", tag="mix")
                for hc in range(2):
                    nc.tensor.transpose(
                        tk[:, hc * P:(hc + 1) * P],
                        qkrot[:, 2 * H + hc * P:2 * H + (hc + 1) * P], idh,
                    )
                nc.vector.tensor_copy(
                    kT_sb[:, :, i * P:(i + 1) * P],
                    tk[:, 0:2 * P].rearrange("p (hc q) -> p hc q", hc=2),
                )

                # ---- E: logits/softmax per head ----
                j0 = max(0, i - NB)
                nblk = i - j0 + 1
                width = nblk * P
                s0 = j0 * P
                pieces = _split_pieces(width)
                encT = work.tile([P, 2 * H], f16, name="encT", tag="encT")
                probs2 = []
                for n in range(2):
                    probs = work.tile([P, 9 * P], f16, name="probs", tag="probs")
                    probs2.append(probs)
                    lacc = work.tile([P, 1], f32, name="lacc", tag="lacc")
                    off = 0
                    for pi, w in enumerate(pieces):
                        pl = ps_mix.tile([P, 512], f32, name="pl", tag="mix")
                        for hc in range(2):
                            nc.tensor.matmul(
                                pl[:, 0:w],
                                qT[:, n * 2 + hc, :],
                                kT_sb[:, hc, s0 + off:s0 + off + w],
                                start=(hc == 0), stop=(hc == 1),
                            )
                        if off == 0 and i >= NB:
                            nc.vector.tensor_add(pl[:, 0:P], pl[:, 0:P], mleft[:])
                        if off + w == width:
                            nc.vector.tensor_add(
                                pl[:, w - P:w], pl[:, w - P:w], mdiag[:]
                            )
                        if pi == 0:
                            acc_t = lacc
                        else:
                            acc_t = work.tile([P, 1], f32, name="lpart",
                                              tag="lpart")
                        nc.scalar.activation(
                            probs[:, off:off + w], pl[:, 0:w], ACTF.Exp,
                            bias=eshift_t[:], accum_out=acc_t[:],
                        )
                        if pi > 0:
                            nc.vector.tensor_add(lacc[:], lacc[:], acc_t[:])
                        off += w
                    rl = work.tile([P, 1], f32, name="rl", tag="rl")
                    nc.vector.reciprocal(rl[:], lacc[:])
                    nc.vector.tensor_scalar_mul(
                        probs[:, 0:width], probs[:, 0:width], rl[:]
                    )
                # ---- F: joint-head PV ----
                # transpose probs blocks for both heads (2 blocks x 2 heads
                # per PSUM bank); pT2[m][n] = probs_n[:, block g0+m].T
                pT2s = []
                for g0 in range(0, nblk, 2):
                    gw = min(2, nblk - g0)
                    ptp = ps_mix.tile([P, 512], f16, name="ptp", tag="mix")
                    for m in range(gw):
                        for n in range(2):
                            nc.tensor.transpose(
                                ptp[:, (m * 2 + n) * P:(m * 2 + n + 1) * P],
                                probs2[n][:, (g0 + m) * P:(g0 + m + 1) * P],
                                idh,
                            )
                    pT2 = work.tile([P, 2, 2, P], f16, name="pT2", tag="pT2",
                                    bufs=6)
                    nc.vector.tensor_copy(
                        pT2[:].rearrange("p a b q -> p (a b q)")[:, 0:gw * 2 * P],
                        ptp[:, 0:gw * 2 * P],
                    )
                    pT2s.append(pT2)
                # encT[h_in, (n, hc) blocks of 128 t-cols]
                for hc in range(2):
                    enc = ps_enc.tile([P, 2 * P], f32, name="enc", tag="enc")
                    for jj in range(nblk):
                        nc.tensor.matmul(
                            enc[:],
                            v_sb[:, j0 + jj, hc * P:(hc + 1) * P],
                            pT2s[jj // 2][:, jj % 2, :, :],
                            start=(jj == 0), stop=(jj == nblk - 1),
                        )
                    # enc cols = (n0 q | n1 q) -> encT blocks hc, 2+hc
                    encT4 = encT[:].rearrange("p (n c q) -> p n c q", n=2, c=2)
                    nc.vector.tensor_copy(
                        encT4[:, :, hc, :],
                        enc[:].rearrange("p (n q) -> p n q", n=2),
                    )

                # ---- H: out projection ----
                out_sb = work.tile([P, D], f32, name="out_sb", tag="out_sb")
                for dq in range(4):
                    po = ps_mix.tile([P, 512], f32, name="po", tag="mix")
                    for hh in range(4):
                        n, hc = divmod(hh, 2)
                        nc.tensor.matmul(
                            po[:],
                            encT[:, hh * P:(hh + 1) * P],
                            wo_sb[:, n, hc, dq * 512:(dq + 1) * 512],
                            start=(hh == 0), stop=(hh == 3),
                        )
                    nc.vector.tensor_copy(
                        out_sb[:, dq * 512:(dq + 1) * 512], po[:]
                    )
                nc.scalar.dma_start(out_d[i * P:(i + 1) * P, :], out_sb[:])

    nc.compile()
    return nc


def get_nc():
    if "nc" not in _CACHE:
        _CACHE["nc"] = _build_nc()
    return _CACHE["nc"]


def make_in_maps(x, segment_pos, q_w, kv_w, o_w, q_scale, k_scale):
    frac = 2.0 * np.arange(H // 2, dtype=np.float32) / np.float32(H)
    timescale = (ROPE_BASE ** frac).astype(np.float32)
    in_maps = []
    for c in range(NCORES):
        b, kv = divmod(c, 4)
        pos = segment_pos[b].astype(np.float32)
        sinusoid = pos[:, None] / timescale[None, :]
        in_maps.append({
            "identh": np.eye(P, dtype=np.float16),
            "x": np.ascontiguousarray(x[b].T, dtype=np.float16),
            "wq": np.ascontiguousarray(q_w[2 * kv:2 * kv + 2], dtype=np.float16),
            "wk": np.ascontiguousarray(kv_w[0, kv], dtype=np.float16),
            "wv": np.ascontiguousarray(kv_w[1, kv], dtype=np.float16),
            "wo": np.ascontiguousarray(o_w[2 * kv:2 * kv + 2], dtype=np.float16),
            "cos": np.cos(sinusoid).astype(np.float16),
            "sin": np.sin(sinusoid).astype(np.float16),
            "qs": (q_scale.astype(np.float32) * np.float32(SCALE)),
            "ks": k_scale.astype(np.float32),
        })
    return in_maps


def kernel(x, segment_pos, attn_mask, q_w, kv_w, o_w, q_scale, k_scale,
           _trace=False, _tmpdir=None):
    from concourse.bass_utils import run_bass_kernel_spmd

    nc = get_nc()
    in_maps = make_in_maps(x, segment_pos, q_w, kv_w, o_w, q_scale, k_scale)
    res = run_bass_kernel_spmd(
        nc, in_maps, core_ids=list(range(NCORES)),
        trace=_trace, tmpdir=_tmpdir,
    )
    out = np.zeros((B, T, D), dtype=np.float32)
    for c in range(NCORES):
        out[c // 4] += res.results[c]["out"]
    if _trace:
        _CACHE["last_result"] = res
    return out
